# revision 8
# baseline (speedup 1.0000x reference)
"""Trainium2 Bass kernel: scatter-add of table rows into a voxel grid.

Computes out[cell] += table[row] for ~1M (cell, row) events, out shape
[B*W*H*L, D] = [131072, 256] fp32.

The limiters on this part are SWDGE descriptor generation (~2.6ns/desc
serialized on the Pool engine) and SDMA byte throughput (~300GB/s under
the activity throttle). Both scale with gather slots, so events within
a tile are row-sorted and pair-matched (maximum disjoint matching on
the sorted path, row gap < 8): a pair is fetched by ONE 1KB descriptor
from the host-built augmented table TDALL[d*4096+a] = [table[a],
table[a+d]] (32MB HBM). Leftover singles (~3%) use separate 512B-elem
gather calls against a zero-extended table copy. Pair-chunk padding
points at an always-zero TDALL row, single padding at the zero row.

Device per pair-chunk (128 slots = up to 256 events): one 128x1KB
dma_gather, two bf16 one-hot builds (cells of first/second events, -1
for dead lanes), two PE matmuls accumulating into the tile's PSUM
bank; single-chunks do one of each. PSUM is downcast to bf16, written
partition-major, reassembled + upcast on host.
"""

import numpy as np
import ml_dtypes

B, W, H, L, D = 4, 32, 32, 32, 256
NCELLS = B * W * H * L          # 131072
TROWS = 4096
NCORES = 8
TPC = NCELLS // 128 // NCORES   # tile positions per core: 128
NDELTA = 8                      # pair row-gap range [0, 7]
PADIDX = NDELTA * TROWS - 1     # 32767: unreachable (a+7>4095), zeroed
GIDX = 1024                     # slots per dma_gather call
GCH = GIDX // 128               # chunks per gather call: 8
NSEG = 8                        # rows_w load segments (early gather start)
OHB = 8                         # one-hot builds batched per DVE op
OB = 8                          # output tiles batched per DMA

_compiled = {}


def _build(key):
    import concourse.tile as tile
    from concourse import bacc, mybir

    Sp, Ss = key
    f32, bf16, i16 = mybir.dt.float32, mybir.dt.bfloat16, mybir.dt.int16
    nchp, nchs = int(sum(Sp)), int(sum(Ss))
    assert nchp % GCH == 0 and nchs % GCH == 0
    ncp, ncs = nchp // GCH, nchs // GCH    # pair / single gather calls

    nc = bacc.Bacc("TRN2", target_bir_lowering=False, debug=False,
                   num_devices=NCORES, num_swdge_queues=4)
    tdall = nc.dram_tensor("tdall", [NDELTA * TROWS, 2 * D], bf16,
                           kind="ExternalInput")
    tabz = nc.dram_tensor("tabz", [TROWS + 1, D], bf16, kind="ExternalInput")
    rows_wp = nc.dram_tensor("rows_wp", [128, ncp * (GIDX // 16)], i16,
                             kind="ExternalInput")
    rows_ws = nc.dram_tensor("rows_ws", [128, max(ncs, 1) * (GIDX // 16)],
                             i16, kind="ExternalInput")
    lrelp = nc.dram_tensor("lrelp", [128, 2, nchp], bf16,
                           kind="ExternalInput")
    lrels = nc.dram_tensor("lrels", [128, max(nchs, 1)], bf16,
                           kind="ExternalInput")
    out = nc.dram_tensor("out", [128, TPC, D], bf16, kind="ExternalOutput")

    with tile.TileContext(nc) as tc:
        with tc.tile_pool(name="const", bufs=1) as constp, \
             tc.tile_pool(name="rows", bufs=2 * NSEG) as rowsp, \
             tc.tile_pool(name="lrelb", bufs=2 * NSEG) as lrelbp, \
             tc.tile_pool(name="gbuf", bufs=9) as gpool, \
             tc.tile_pool(name="gsbuf", bufs=4) as gspool, \
             tc.tile_pool(name="oh", bufs=12) as ohpool, \
             tc.tile_pool(name="psum", bufs=8, space="PSUM") as pspool, \
             tc.tile_pool(name="stage", bufs=4) as stpool:
            def seg_load(pool, dram, cols, nseg, dt, sub=None):
                tiles = []
                seg = -(-cols // nseg)
                seg += (-seg) % (GIDX // 16)   # call windows never straddle
                for si in range(nseg):
                    lo, hi = si * seg, min((si + 1) * seg, cols)
                    if lo >= hi:
                        tiles.append(None)
                        continue
                    if sub is None:
                        t = pool.tile([128, hi - lo], dt)
                        nc.sync.dma_start(t[:], dram[:, lo:hi])
                    else:
                        t = pool.tile([128, sub, hi - lo], dt)
                        nc.sync.dma_start(t[:], dram[:, :, lo:hi])
                    tiles.append(t)
                return tiles, seg

            rp_sb, rp_seg = seg_load(rowsp, rows_wp, ncp * (GIDX // 16),
                                     NSEG, i16)
            rs_sb, rs_seg = seg_load(rowsp, rows_ws,
                                     max(ncs, 1) * (GIDX // 16), 2, i16)
            # lrel segments aligned to OHB batches
            lpseg = -(-nchp // NSEG)
            lpseg += (-lpseg) % OHB
            lp_sb = []
            for si in range(NSEG):
                lo, hi = si * lpseg, min((si + 1) * lpseg, nchp)
                if lo >= hi:
                    lp_sb.append(None)
                    continue
                t = lrelbp.tile([128, 2, hi - lo], bf16)
                nc.sync.dma_start(t[:], lrelp[:, :, lo:hi])
                lp_sb.append(t)
            lsseg = max(nchs, 1)
            ls_sb = lrelbp.tile([128, lsseg], bf16)
            nc.sync.dma_start(ls_sb[:], lrels[:, :])

            iota_t = constp.tile([128, OHB, 128], bf16)
            nc.gpsimd.iota(iota_t[:], pattern=[[0, OHB], [1, 128]], base=0,
                           channel_multiplier=0,
                           allow_small_or_imprecise_dtypes=True)

            gtp = gts = oha = ohb = ohs = st = None
            cp = 0      # global pair-chunk counter
            cs = 0      # global single-chunk counter
            qn = 0      # round-robin queue over all gather calls
            for t in range(TPC):
                ps = pspool.tile([128, D], f32, space="PSUM")
                Kp, Ks = int(Sp[t]), int(Ss[t])
                nmm = 2 * Kp + Ks
                mm = 0
                for j in range(Kp):
                    if cp % GCH == 0:
                        ci = cp // GCH
                        gtp = gpool.tile([128, GCH, 2 * D], bf16)
                        seg = rp_sb[ci * (GIDX // 16) // rp_seg]
                        so = ci * (GIDX // 16) - (
                            ci * (GIDX // 16) // rp_seg) * rp_seg
                        nc.gpsimd.dma_gather(
                            gtp[:], tdall[:], seg[:, so:so + (GIDX // 16)],
                            GIDX, GIDX, 2 * D, queue_num=qn % 4)
                        qn += 1
                    if cp % OHB == 0:
                        lseg_t = lp_sb[cp // lpseg]
                        lo = cp - (cp // lpseg) * lpseg
                        nb = min(OHB, nchp - cp, lpseg - lo)
                        oha = ohpool.tile([128, OHB, 128], bf16)
                        ohb = ohpool.tile([128, OHB, 128], bf16)
                        nc.vector.tensor_tensor(
                            out=oha[:, :nb, :],
                            in0=lseg_t[:, 0, lo:lo + nb, None].to_broadcast(
                                [128, nb, 128]),
                            in1=iota_t[:, :nb, :],
                            op=mybir.AluOpType.is_equal)
                        nc.vector.tensor_tensor(
                            out=ohb[:, :nb, :],
                            in0=lseg_t[:, 1, lo:lo + nb, None].to_broadcast(
                                [128, nb, 128]),
                            in1=iota_t[:, :nb, :],
                            op=mybir.AluOpType.is_equal)
                    nc.tensor.matmul(out=ps[:], lhsT=oha[:, cp % OHB, :],
                                     rhs=gtp[:, cp % GCH, 0:D],
                                     start=(mm == 0), stop=(mm == nmm - 1))
                    mm += 1
                    nc.tensor.matmul(out=ps[:], lhsT=ohb[:, cp % OHB, :],
                                     rhs=gtp[:, cp % GCH, D:2 * D],
                                     start=(mm == 0), stop=(mm == nmm - 1))
                    mm += 1
                    cp += 1
                for j in range(Ks):
                    if cs % GCH == 0:
                        ci = cs // GCH
                        gts = gspool.tile([128, GCH, D], bf16)
                        seg = rs_sb[ci * (GIDX // 16) // rs_seg]
                        so = ci * (GIDX // 16) - (
                            ci * (GIDX // 16) // rs_seg) * rs_seg
                        nc.gpsimd.dma_gather(
                            gts[:], tabz[:], seg[:, so:so + (GIDX // 16)],
                            GIDX, GIDX, D, queue_num=qn % 4)
                        qn += 1
                    if cs % OHB == 0:
                        nb = min(OHB, nchs - cs)
                        ohs = ohpool.tile([128, OHB, 128], bf16)
                        nc.vector.tensor_tensor(
                            out=ohs[:, :nb, :],
                            in0=ls_sb[:, cs:cs + nb, None].to_broadcast(
                                [128, nb, 128]),
                            in1=iota_t[:, :nb, :],
                            op=mybir.AluOpType.is_equal)
                    nc.tensor.matmul(out=ps[:], lhsT=ohs[:, cs % OHB, :],
                                     rhs=gts[:, cs % GCH, :],
                                     start=(mm == 0), stop=(mm == nmm - 1))
                    mm += 1
                    cs += 1
                if t % OB == 0:
                    st = stpool.tile([128, OB, D], bf16)
                nc.any.tensor_copy(st[:, t % OB, :], ps[:])
                if t % OB == OB - 1:
                    t0 = t - (OB - 1)
                    nc.sync.dma_start(out[:, t0:t0 + OB, :], st[:])
            assert cp == nchp and cs == nchs
    nc.compile()
    return nc


def _match_tile(r, l):
    """Maximum disjoint matching of consecutive row-sorted events with
    gap < NDELTA (greedy over runs of usable edges = optimal on a path).

    Returns (pair_idx, pair_cA, pair_cB, single_row, single_cell)."""
    n = len(r)
    if n == 0:
        z = np.zeros(0, np.int64)
        return z, z, z, z, z
    gap = r[1:] - r[:-1]
    usable = gap < NDELTA
    # greedy = optimal on a path: within each run of consecutive usable
    # edges, take edges at even run-relative positions (vectorized).
    if n > 1:
        idxs_ = np.arange(n - 1)
        prev_brk = np.maximum.accumulate(np.where(~usable, idxs_, -1))
        rel = idxs_ - (prev_brk + 1)
        take = usable & (rel % 2 == 0)
    else:
        take = np.zeros(0, np.bool_)
    paired_first = np.zeros(n, np.bool_)
    if n > 1:
        paired_first[:-1] = take
    paired_second = np.zeros(n, np.bool_)
    if n > 1:
        paired_second[1:] = take
    single = ~(paired_first | paired_second)
    pf = np.nonzero(paired_first)[0]
    idx = (r[pf + 1] - r[pf]) * TROWS + r[pf]
    o = np.argsort(idx, kind="stable")
    sr = np.nonzero(single)[0]
    return idx[o], l[pf][o], l[pf + 1][o], np.sort(r[sr]), l[sr][np.argsort(r[sr], kind="stable")]


def _marshal(event_cell, event_row):
    ecell = np.asarray(event_cell).astype(np.int64)
    erow = np.asarray(event_row).astype(np.int64)
    order = np.argsort(ecell, kind="stable")
    scell = ecell[order]
    srow = erow[order].astype(np.int64)

    ntiles = NCELLS // 128
    bounds = np.searchsorted(scell, np.arange(ntiles + 1) * 128)
    counts = np.diff(bounds)

    tiles = []
    for t in range(ntiles):
        s, n = int(bounds[t]), int(counts[t])
        rr, ll = srow[s:s + n], scell[s:s + n] & 127
        ro = np.argsort(rr, kind="stable")
        tiles.append(_match_tile(rr[ro], ll[ro]))
    kp = np.array([-(-len(ts[0]) // 128) for ts in tiles])
    ks = np.array([-(-len(ts[3]) // 128) for ts in tiles])
    load = kp * 2 + ks                       # matmul count as balance proxy

    deal = np.argsort(-load, kind="stable")
    assign = [[] for _ in range(NCORES)]
    for rank, t in enumerate(deal):
        rr = rank % (2 * NCORES)
        cidx = rr if rr < NCORES else 2 * NCORES - 1 - rr
        assign[cidx].append(int(t))
    pos_tiles = [sorted(ts, key=lambda t: (-load[t], -kp[t], t))
                 for ts in assign]
    Sp = np.max(np.stack([[kp[t] for t in ts] for ts in pos_tiles]), axis=0)
    Ss = np.max(np.stack([[ks[t] for t in ts] for ts in pos_tiles]), axis=0)
    Sp, Ss = Sp.astype(np.int64), Ss.astype(np.int64)
    for t in range(TPC):                     # no empty psum accumulations
        if Sp[t] == 0 and Ss[t] == 0:
            Ss[t] = 1
    Sp[-1] += (-int(Sp.sum())) % GCH
    Ss[-1] += (-int(Ss.sum())) % GCH
    nchp, nchs = int(Sp.sum()), int(Ss.sum())
    offp = np.concatenate([[0], np.cumsum(Sp)])
    offs = np.concatenate([[0], np.cumsum(Ss)])

    def wrap(v):
        wr = v.reshape(-1, GIDX).reshape(-1, GIDX // 16, 16)
        wr = wr.transpose(0, 2, 1).reshape(-1, 16, GIDX // 16)
        wr = np.concatenate(list(wr), axis=1)
        return np.ascontiguousarray(np.tile(wr, (8, 1)))

    in_maps = []
    for cidx in range(NCORES):
        slp = np.full(nchp * 128, PADIDX, np.int16)
        sls = np.full(max(nchs, 1) * 128, TROWS, np.int16)
        lrp = np.full((2, nchp * 128), -1.0, np.float32)
        lrs = np.full(max(nchs, 1) * 128, -1.0, np.float32)
        for p, t in enumerate(pos_tiles[cidx]):
            pidx, pa, pb, srows, scells = tiles[t]
            bp = int(offp[p]) * 128
            n = len(pidx)
            slp[bp:bp + n] = pidx.astype(np.int16)
            lrp[0, bp:bp + n] = pa
            lrp[1, bp:bp + n] = pb
            bs = int(offs[p]) * 128
            m = len(srows)
            sls[bs:bs + m] = srows.astype(np.int16)
            lrs[bs:bs + m] = scells
        lcp = lrp.reshape(2, nchp, 128).transpose(2, 0, 1)
        lcs = lrs.reshape(-1, 128).T
        in_maps.append({
            "rows_wp": wrap(slp),
            "rows_ws": wrap(sls),
            "lrelp": np.ascontiguousarray(lcp.astype(ml_dtypes.bfloat16)),
            "lrels": np.ascontiguousarray(lcs.astype(ml_dtypes.bfloat16)),
        })
    return in_maps, (tuple(int(x) for x in Sp), tuple(int(x) for x in Ss)), \
        pos_tiles


def kernel(table, event_cell, event_row, _want_trace=False):
    from concourse.bass_utils import run_bass_kernel_spmd

    tabbf = np.asarray(table, dtype=np.float32).astype(ml_dtypes.bfloat16)
    td = np.empty((NDELTA, TROWS, 2 * D), dtype=ml_dtypes.bfloat16)
    ar = np.arange(TROWS)
    for dlt in range(NDELTA):
        td[dlt, :, :D] = tabbf
        td[dlt, :, D:] = tabbf[np.minimum(ar + dlt, TROWS - 1)]
    td = td.reshape(NDELTA * TROWS, 2 * D)
    td[PADIDX] = 0
    td = np.ascontiguousarray(td)
    tz = np.zeros((TROWS + 1, D), dtype=ml_dtypes.bfloat16)
    tz[:TROWS] = tabbf

    in_maps, key, pos_tiles = _marshal(event_cell, event_row)
    for m in in_maps:
        m["tdall"] = td
        m["tabz"] = tz

    if key not in _compiled:
        _compiled[key] = _build(key)
    nc = _compiled[key]

    kw = {"trace": True} if _want_trace else {}
    res = run_bass_kernel_spmd(nc, in_maps, core_ids=list(range(NCORES)), **kw)
    full = np.empty((NCELLS // 128, 128, D), np.float32)
    for cidx in range(NCORES):
        co = np.asarray(res.results[cidx]["out"]).astype(np.float32)
        full[np.array(pos_tiles[cidx])] = co.transpose(1, 0, 2)
    out = full.reshape(B, W, H, L, D)
    if _want_trace:
        return out, res
    return out


# revision 9
# speedup vs baseline: 1.0106x; 1.0106x over previous
"""Trainium2 Bass kernel: scatter-add of table rows into a voxel grid.

Computes out[cell] += table[row] for ~1M (cell, row) events, out shape
[B*W*H*L, D] = [131072, 256] fp32.

The limiters on this part are SWDGE descriptor generation (~2.6ns/desc
serialized on the Pool engine) and SDMA byte throughput (~300GB/s under
the activity throttle). Both scale with gather slots, so events within
a tile are row-sorted and pair-matched (maximum disjoint matching on
the sorted path, row gap < 8): a pair is fetched by ONE 1KB descriptor
from the host-built augmented table TDALL[d*4096+a] = [table[a],
table[a+d]] (32MB HBM). Leftover singles (~3%) use separate 512B-elem
gather calls against a zero-extended table copy. Pair-chunk padding
points at an always-zero TDALL row, single padding at the zero row.

Device per pair-chunk (128 slots = up to 256 events): one 128x1KB
dma_gather, two bf16 one-hot builds (cells of first/second events, -1
for dead lanes), two PE matmuls accumulating into the tile's PSUM
bank; single-chunks do one of each. PSUM is downcast to bf16, written
partition-major, reassembled + upcast on host.
"""

import numpy as np
import ml_dtypes

B, W, H, L, D = 4, 32, 32, 32, 256
NCELLS = B * W * H * L          # 131072
TROWS = 4096
NCORES = 8
TPC = NCELLS // 128 // NCORES   # tile positions per core: 128
NDELTA = 8                      # pair row-gap range [0, 7]
PADIDX = NDELTA * TROWS - 1     # 32767: unreachable (a+7>4095), zeroed
GIDX = 1024                     # slots per dma_gather call
GCH = GIDX // 128               # chunks per gather call: 8
NSEG = 8                        # rows_w load segments (early gather start)
OHB = 8                         # one-hot builds batched per DVE op
OB = 8                          # output tiles batched per DMA

_compiled = {}


def _build(key):
    import concourse.tile as tile
    from concourse import bacc, mybir

    Sp, Ss = key
    f32, bf16, i16 = mybir.dt.float32, mybir.dt.bfloat16, mybir.dt.int16
    f8 = mybir.dt.float8e4
    nchp, nchs = int(sum(Sp)), int(sum(Ss))
    assert nchp % GCH == 0 and nchs % GCH == 0
    ncp, ncs = nchp // GCH, nchs // GCH    # pair / single gather calls

    nc = bacc.Bacc("TRN2", target_bir_lowering=False, debug=False,
                   num_devices=NCORES, num_swdge_queues=4)
    tdall = nc.dram_tensor("tdall", [NDELTA * TROWS, 2 * D], bf16,
                           kind="ExternalInput")
    tabz = nc.dram_tensor("tabz", [TROWS + 1, D], bf16, kind="ExternalInput")
    rows_wp = nc.dram_tensor("rows_wp", [128, ncp * (GIDX // 16)], i16,
                             kind="ExternalInput")
    rows_ws = nc.dram_tensor("rows_ws", [128, max(ncs, 1) * (GIDX // 16)],
                             i16, kind="ExternalInput")
    lrelp = nc.dram_tensor("lrelp", [128, 2, nchp], bf16,
                           kind="ExternalInput")
    lrels = nc.dram_tensor("lrels", [128, max(nchs, 1)], bf16,
                           kind="ExternalInput")
    out = nc.dram_tensor("out", [128, TPC, D], bf16, kind="ExternalOutput")

    with tile.TileContext(nc) as tc:
        with tc.tile_pool(name="const", bufs=1) as constp, \
             tc.tile_pool(name="rows", bufs=2 * NSEG) as rowsp, \
             tc.tile_pool(name="lrelb", bufs=2 * NSEG) as lrelbp, \
             tc.tile_pool(name="gbuf", bufs=9) as gpool, \
             tc.tile_pool(name="gsbuf", bufs=4) as gspool, \
             tc.tile_pool(name="oh", bufs=12) as ohpool, \
             tc.tile_pool(name="psum", bufs=8, space="PSUM") as pspool, \
             tc.tile_pool(name="stage", bufs=4) as stpool:
            def seg_load(pool, dram, cols, nseg, dt, sub=None):
                tiles = []
                seg = -(-cols // nseg)
                seg += (-seg) % (GIDX // 16)   # call windows never straddle
                for si in range(nseg):
                    lo, hi = si * seg, min((si + 1) * seg, cols)
                    if lo >= hi:
                        tiles.append(None)
                        continue
                    if sub is None:
                        t = pool.tile([128, hi - lo], dt)
                        nc.sync.dma_start(t[:], dram[:, lo:hi])
                    else:
                        t = pool.tile([128, sub, hi - lo], dt)
                        nc.sync.dma_start(t[:], dram[:, :, lo:hi])
                    tiles.append(t)
                return tiles, seg

            rp_sb, rp_seg = seg_load(rowsp, rows_wp, ncp * (GIDX // 16),
                                     NSEG, i16)
            rs_sb, rs_seg = seg_load(rowsp, rows_ws,
                                     max(ncs, 1) * (GIDX // 16), 2, i16)
            # lrel segments aligned to OHB batches
            lpseg = -(-nchp // NSEG)
            lpseg += (-lpseg) % OHB
            lp_sb = []
            for si in range(NSEG):
                lo, hi = si * lpseg, min((si + 1) * lpseg, nchp)
                if lo >= hi:
                    lp_sb.append(None)
                    continue
                t = lrelbp.tile([128, 2, hi - lo], bf16)
                nc.sync.dma_start(t[:], lrelp[:, :, lo:hi])
                lp_sb.append(t)
            lsseg = max(nchs, 1)
            ls_sb = lrelbp.tile([128, lsseg], bf16)
            nc.sync.dma_start(ls_sb[:], lrels[:, :])

            iota_t = constp.tile([128, OHB, 128], bf16)
            nc.gpsimd.iota(iota_t[:], pattern=[[0, OHB], [1, 128]], base=0,
                           channel_multiplier=0,
                           allow_small_or_imprecise_dtypes=True)

            gtp = gts = oha = ohb = ohs = st = None
            cp = 0      # global pair-chunk counter
            cs = 0      # global single-chunk counter
            qn = 0      # round-robin queue over all gather calls
            for t in range(TPC):
                ps = pspool.tile([128, D], f32, space="PSUM")
                Kp, Ks = int(Sp[t]), int(Ss[t])
                nmm = 2 * Kp + Ks
                mm = 0
                for j in range(Kp):
                    if cp % GCH == 0:
                        ci = cp // GCH
                        gtp = gpool.tile([128, GCH, 2 * D], bf16)
                        seg = rp_sb[ci * (GIDX // 16) // rp_seg]
                        so = ci * (GIDX // 16) - (
                            ci * (GIDX // 16) // rp_seg) * rp_seg
                        nc.gpsimd.dma_gather(
                            gtp[:], tdall[:], seg[:, so:so + (GIDX // 16)],
                            GIDX, GIDX, 2 * D, queue_num=qn % 4)
                        qn += 1
                    if cp % OHB == 0:
                        lseg_t = lp_sb[cp // lpseg]
                        lo = cp - (cp // lpseg) * lpseg
                        nb = min(OHB, nchp - cp, lpseg - lo)
                        oha = ohpool.tile([128, OHB, 128], f8)
                        ohb = ohpool.tile([128, OHB, 128], f8)
                        nc.vector.tensor_tensor(
                            out=oha[:, :nb, :],
                            in0=lseg_t[:, 0, lo:lo + nb, None].to_broadcast(
                                [128, nb, 128]),
                            in1=iota_t[:, :nb, :],
                            op=mybir.AluOpType.is_equal)
                        nc.vector.tensor_tensor(
                            out=ohb[:, :nb, :],
                            in0=lseg_t[:, 1, lo:lo + nb, None].to_broadcast(
                                [128, nb, 128]),
                            in1=iota_t[:, :nb, :],
                            op=mybir.AluOpType.is_equal)
                    nc.tensor.matmul(out=ps[:], lhsT=oha[:, cp % OHB, :],
                                     rhs=gtp[:, cp % GCH, 0:D],
                                     start=(mm == 0), stop=(mm == nmm - 1))
                    mm += 1
                    nc.tensor.matmul(out=ps[:], lhsT=ohb[:, cp % OHB, :],
                                     rhs=gtp[:, cp % GCH, D:2 * D],
                                     start=(mm == 0), stop=(mm == nmm - 1))
                    mm += 1
                    cp += 1
                for j in range(Ks):
                    if cs % GCH == 0:
                        ci = cs // GCH
                        gts = gspool.tile([128, GCH, D], bf16)
                        seg = rs_sb[ci * (GIDX // 16) // rs_seg]
                        so = ci * (GIDX // 16) - (
                            ci * (GIDX // 16) // rs_seg) * rs_seg
                        nc.gpsimd.dma_gather(
                            gts[:], tabz[:], seg[:, so:so + (GIDX // 16)],
                            GIDX, GIDX, D, queue_num=qn % 4)
                        qn += 1
                    if cs % OHB == 0:
                        nb = min(OHB, nchs - cs)
                        ohs = ohpool.tile([128, OHB, 128], f8)
                        nc.vector.tensor_tensor(
                            out=ohs[:, :nb, :],
                            in0=ls_sb[:, cs:cs + nb, None].to_broadcast(
                                [128, nb, 128]),
                            in1=iota_t[:, :nb, :],
                            op=mybir.AluOpType.is_equal)
                    nc.tensor.matmul(out=ps[:], lhsT=ohs[:, cs % OHB, :],
                                     rhs=gts[:, cs % GCH, :],
                                     start=(mm == 0), stop=(mm == nmm - 1))
                    mm += 1
                    cs += 1
                if t % OB == 0:
                    st = stpool.tile([128, OB, D], bf16)
                nc.any.tensor_copy(st[:, t % OB, :], ps[:])
                if t % OB == OB - 1:
                    t0 = t - (OB - 1)
                    nc.sync.dma_start(out[:, t0:t0 + OB, :], st[:])
            assert cp == nchp and cs == nchs
    nc.compile()
    return nc


def _match_tile(r, l):
    """Maximum disjoint matching of consecutive row-sorted events with
    gap < NDELTA (greedy over runs of usable edges = optimal on a path).

    Returns (pair_idx, pair_cA, pair_cB, single_row, single_cell)."""
    n = len(r)
    if n == 0:
        z = np.zeros(0, np.int64)
        return z, z, z, z, z
    gap = r[1:] - r[:-1]
    usable = gap < NDELTA
    # greedy = optimal on a path: within each run of consecutive usable
    # edges, take edges at even run-relative positions (vectorized).
    if n > 1:
        idxs_ = np.arange(n - 1)
        prev_brk = np.maximum.accumulate(np.where(~usable, idxs_, -1))
        rel = idxs_ - (prev_brk + 1)
        take = usable & (rel % 2 == 0)
    else:
        take = np.zeros(0, np.bool_)
    paired_first = np.zeros(n, np.bool_)
    if n > 1:
        paired_first[:-1] = take
    paired_second = np.zeros(n, np.bool_)
    if n > 1:
        paired_second[1:] = take
    single = ~(paired_first | paired_second)
    pf = np.nonzero(paired_first)[0]
    idx = (r[pf + 1] - r[pf]) * TROWS + r[pf]
    o = np.argsort(idx, kind="stable")
    sr = np.nonzero(single)[0]
    return idx[o], l[pf][o], l[pf + 1][o], np.sort(r[sr]), l[sr][np.argsort(r[sr], kind="stable")]


def _marshal(event_cell, event_row):
    ecell = np.asarray(event_cell).astype(np.int64)
    erow = np.asarray(event_row).astype(np.int64)
    order = np.argsort(ecell, kind="stable")
    scell = ecell[order]
    srow = erow[order].astype(np.int64)

    ntiles = NCELLS // 128
    bounds = np.searchsorted(scell, np.arange(ntiles + 1) * 128)
    counts = np.diff(bounds)

    tiles = []
    for t in range(ntiles):
        s, n = int(bounds[t]), int(counts[t])
        rr, ll = srow[s:s + n], scell[s:s + n] & 127
        ro = np.argsort(rr, kind="stable")
        tiles.append(_match_tile(rr[ro], ll[ro]))
    kp = np.array([-(-len(ts[0]) // 128) for ts in tiles])
    ks = np.array([-(-len(ts[3]) // 128) for ts in tiles])
    load = kp * 2 + ks                       # matmul count as balance proxy

    deal = np.argsort(-load, kind="stable")
    assign = [[] for _ in range(NCORES)]
    for rank, t in enumerate(deal):
        rr = rank % (2 * NCORES)
        cidx = rr if rr < NCORES else 2 * NCORES - 1 - rr
        assign[cidx].append(int(t))
    pos_tiles = [sorted(ts, key=lambda t: (-load[t], -kp[t], t))
                 for ts in assign]
    # interleave positions (same permutation on every core): spreads the
    # small-tile bursts of out-DMAs across the whole run instead of the tail
    perm = []
    lohalf = list(range(TPC // 2))
    hihalf = list(range(TPC // 2, TPC))
    for a, b in zip(lohalf, reversed(hihalf)):
        perm += [a, b]
    pos_tiles = [[ts[i] for i in perm] for ts in pos_tiles]
    Sp = np.max(np.stack([[kp[t] for t in ts] for ts in pos_tiles]), axis=0)
    Ss = np.max(np.stack([[ks[t] for t in ts] for ts in pos_tiles]), axis=0)
    Sp, Ss = Sp.astype(np.int64), Ss.astype(np.int64)
    for t in range(TPC):                     # no empty psum accumulations
        if Sp[t] == 0 and Ss[t] == 0:
            Ss[t] = 1
    Sp[-1] += (-int(Sp.sum())) % GCH
    Ss[-1] += (-int(Ss.sum())) % GCH
    nchp, nchs = int(Sp.sum()), int(Ss.sum())
    offp = np.concatenate([[0], np.cumsum(Sp)])
    offs = np.concatenate([[0], np.cumsum(Ss)])

    def wrap(v):
        wr = v.reshape(-1, GIDX).reshape(-1, GIDX // 16, 16)
        wr = wr.transpose(0, 2, 1).reshape(-1, 16, GIDX // 16)
        wr = np.concatenate(list(wr), axis=1)
        return np.ascontiguousarray(np.tile(wr, (8, 1)))

    in_maps = []
    for cidx in range(NCORES):
        slp = np.full(nchp * 128, PADIDX, np.int16)
        sls = np.full(max(nchs, 1) * 128, TROWS, np.int16)
        lrp = np.full((2, nchp * 128), -1.0, np.float32)
        lrs = np.full(max(nchs, 1) * 128, -1.0, np.float32)
        for p, t in enumerate(pos_tiles[cidx]):
            pidx, pa, pb, srows, scells = tiles[t]
            bp = int(offp[p]) * 128
            n = len(pidx)
            slp[bp:bp + n] = pidx.astype(np.int16)
            lrp[0, bp:bp + n] = pa
            lrp[1, bp:bp + n] = pb
            bs = int(offs[p]) * 128
            m = len(srows)
            sls[bs:bs + m] = srows.astype(np.int16)
            lrs[bs:bs + m] = scells
        lcp = lrp.reshape(2, nchp, 128).transpose(2, 0, 1)
        lcs = lrs.reshape(-1, 128).T
        in_maps.append({
            "rows_wp": wrap(slp),
            "rows_ws": wrap(sls),
            "lrelp": np.ascontiguousarray(lcp.astype(ml_dtypes.bfloat16)),
            "lrels": np.ascontiguousarray(lcs.astype(ml_dtypes.bfloat16)),
        })
    return in_maps, (tuple(int(x) for x in Sp), tuple(int(x) for x in Ss)), \
        pos_tiles


def kernel(table, event_cell, event_row, _want_trace=False):
    from concourse.bass_utils import run_bass_kernel_spmd

    tabbf = np.asarray(table, dtype=np.float32).astype(ml_dtypes.bfloat16)
    td = np.empty((NDELTA, TROWS, 2 * D), dtype=ml_dtypes.bfloat16)
    ar = np.arange(TROWS)
    for dlt in range(NDELTA):
        td[dlt, :, :D] = tabbf
        td[dlt, :, D:] = tabbf[np.minimum(ar + dlt, TROWS - 1)]
    td = td.reshape(NDELTA * TROWS, 2 * D)
    td[PADIDX] = 0
    td = np.ascontiguousarray(td)
    tz = np.zeros((TROWS + 1, D), dtype=ml_dtypes.bfloat16)
    tz[:TROWS] = tabbf

    in_maps, key, pos_tiles = _marshal(event_cell, event_row)
    for m in in_maps:
        m["tdall"] = td
        m["tabz"] = tz

    if key not in _compiled:
        _compiled[key] = _build(key)
    nc = _compiled[key]

    kw = {"trace": True} if _want_trace else {}
    res = run_bass_kernel_spmd(nc, in_maps, core_ids=list(range(NCORES)), **kw)
    full = np.empty((NCELLS // 128, 128, D), np.float32)
    for cidx in range(NCORES):
        co = np.asarray(res.results[cidx]["out"]).astype(np.float32)
        full[np.array(pos_tiles[cidx])] = co.transpose(1, 0, 2)
    out = full.reshape(B, W, H, L, D)
    if _want_trace:
        return out, res
    return out


# revision 11
# speedup vs baseline: 1.1833x; 1.1708x over previous
"""Trainium2 Bass kernel: scatter-add of table rows into a voxel grid.

Computes out[cell] += table[row] for ~1M (cell, row) events, out shape
[B*W*H*L, D] = [131072, 256] fp32.

Pairing strategy: the bottleneck is SWDGE descriptor generation on the
Pool engine (~2.6ns/descriptor, one descriptor per gathered event row).
To halve the descriptor count, events within a tile are sorted by table
row and paired on the fixed (even, odd) grid whenever the row gap of
the pair is <= 7. A host-built augmented table TDALL[d*4096 + a] =
[table[a], table[a+d]] (8 deltas x 4096 rows x 1KB = 32MB HBM) lets one
1KB descriptor fetch both rows of a pair. Unpaired events use delta=0
with the second half masked off via a -1 one-hot lane.

Device per pair-chunk (128 slots = up to 256 events): one dma_gather of
128 x 1KB, two fp8 one-hot builds (cells of first/second events), two
matmuls accumulating into the tile's PSUM bank. Output is downcast to
bf16, written partition-major, reassembled + upcast on host.
"""

import numpy as np
import ml_dtypes

B, W, H, L, D = 4, 32, 32, 32, 256
NCELLS = B * W * H * L          # 131072
TROWS = 4096
NCORES = 8
TPC = NCELLS // 128 // NCORES   # tile positions per core: 128
NDELTA = 8                      # pair row-gap range [0, 7]
GIDX = 1024                     # pair-slots per dma_gather call
GCH = GIDX // 128               # pair-chunks per gather call: 8
NSEG = 8                        # rows_w load segments (early gather start)
OHB = 8                         # one-hot builds batched per DVE op
OB = 8                          # output tiles batched per DMA

_compiled = {}


def _build(S):
    import concourse.tile as tile
    from concourse import bacc, mybir

    f32, bf16, i16 = mybir.dt.float32, mybir.dt.bfloat16, mybir.dt.int16
    f8 = mybir.dt.float8e4
    nch = int(sum(S))                    # pair-chunks per core
    assert nch % GCH == 0
    ncalls = nch // GCH
    cps = -(-ncalls // NSEG)             # gather calls per rows_w segment

    nc = bacc.Bacc("TRN2", target_bir_lowering=False, debug=False,
                   num_devices=NCORES, num_swdge_queues=4)
    tdall = nc.dram_tensor("tdall", [NDELTA * TROWS, 2 * D], bf16,
                           kind="ExternalInput")
    rows_w = nc.dram_tensor("rows_w", [128, ncalls * (GIDX // 16)], i16,
                            kind="ExternalInput")
    lrel = nc.dram_tensor("lrel", [128, 2, nch], bf16, kind="ExternalInput")
    out = nc.dram_tensor("out", [128, TPC, D], bf16, kind="ExternalOutput")

    with tile.TileContext(nc) as tc:
        with tc.tile_pool(name="const", bufs=1) as constp, \
             tc.tile_pool(name="rows", bufs=NSEG) as rowsp, \
             tc.tile_pool(name="lrelp", bufs=NSEG) as lrelp, \
             tc.tile_pool(name="gbuf", bufs=12) as gpool, \
             tc.tile_pool(name="oh", bufs=18) as ohpool, \
             tc.tile_pool(name="psum", bufs=8, space="PSUM") as pspool, \
             tc.tile_pool(name="stage", bufs=6) as stpool:
            rows_sb = []
            for si in range(NSEG):
                lo = si * cps * (GIDX // 16)
                hi = min((si + 1) * cps * (GIDX // 16), ncalls * (GIDX // 16))
                if lo >= hi:
                    rows_sb.append(None)
                    continue
                t = rowsp.tile([128, hi - lo], i16)
                nc.sync.dma_start(t[:], rows_w[:, lo:hi])
                rows_sb.append(t)
            lrel_sb = []
            lseg = -(-nch // NSEG)
            lseg += (-lseg) % OHB        # align segments to one-hot batches
            for si in range(NSEG):
                lo, hi = si * lseg, min((si + 1) * lseg, nch)
                if lo >= hi:
                    lrel_sb.append(None)
                    continue
                t = lrelp.tile([128, 2, hi - lo], bf16)
                nc.sync.dma_start(t[:], lrel[:, :, lo:hi])
                lrel_sb.append(t)
            iota_t = constp.tile([128, OHB, 128], bf16)
            nc.gpsimd.iota(iota_t[:], pattern=[[0, OHB], [1, 128]], base=0,
                           channel_multiplier=0,
                           allow_small_or_imprecise_dtypes=True)

            gt = None
            oha = None
            ohb = None
            st = None
            c = 0       # global pair-chunk counter
            for t in range(TPC):
                ps = pspool.tile([128, D], f32, space="PSUM")
                K = int(S[t])
                for j in range(K):
                    if c % GCH == 0:
                        ci = c // GCH
                        gt = gpool.tile([128, GCH, 2 * D], bf16)
                        seg = rows_sb[ci // cps]
                        so = (ci % cps) * (GIDX // 16)
                        nc.gpsimd.dma_gather(
                            gt[:], tdall[:],
                            seg[:, so:so + (GIDX // 16)],
                            GIDX, GIDX, 2 * D, queue_num=ci % 4)
                    if c % OHB == 0:
                        lseg_t = lrel_sb[c // lseg]
                        lo = c - (c // lseg) * lseg
                        nb = min(OHB, nch - c, lseg - lo)
                        oha = ohpool.tile([128, OHB, 128], f8)
                        ohb = ohpool.tile([128, OHB, 128], f8)
                        nc.vector.tensor_tensor(
                            out=oha[:, :nb, :],
                            in0=lseg_t[:, 0, lo:lo + nb, None].to_broadcast(
                                [128, nb, 128]),
                            in1=iota_t[:, :nb, :],
                            op=mybir.AluOpType.is_equal)
                        nc.vector.tensor_tensor(
                            out=ohb[:, :nb, :],
                            in0=lseg_t[:, 1, lo:lo + nb, None].to_broadcast(
                                [128, nb, 128]),
                            in1=iota_t[:, :nb, :],
                            op=mybir.AluOpType.is_equal)
                    nc.tensor.matmul(out=ps[:], lhsT=oha[:, c % OHB, :],
                                     rhs=gt[:, c % GCH, 0:D],
                                     start=(j == 0), stop=False)
                    nc.tensor.matmul(out=ps[:], lhsT=ohb[:, c % OHB, :],
                                     rhs=gt[:, c % GCH, D:2 * D],
                                     start=False, stop=(j == K - 1))
                    c += 1
                if t % OB == 0:
                    st = stpool.tile([128, OB, D], bf16)
                nc.any.tensor_copy(st[:, t % OB, :], ps[:])
                if t % OB == OB - 1:
                    t0 = t - (OB - 1)
                    nc.sync.dma_start(out[:, t0:t0 + OB, :], st[:])
            assert c == nch
    nc.compile()
    return nc


def _pair_tile(r, l):
    """Fixed-grid pairing of one tile's row-sorted events.

    Returns (idx, cellA, cellB) int32 arrays, one entry per slot."""
    n = len(r)
    idxs, ca, cb = [], [], []
    k = 0
    half = n // 2
    if half:
        re, ro = r[0:2 * half:2].astype(np.int64), r[1:2 * half:2].astype(np.int64)
        le, lo_ = l[0:2 * half:2], l[1:2 * half:2]
        gap = ro - re
        ok = gap < NDELTA
        # paired slots
        idxs.append((gap[ok] * TROWS + re[ok]))
        ca.append(le[ok])
        cb.append(lo_[ok])
        # broken pairs -> two singles each
        for rr, ll in ((re[~ok], le[~ok]), (ro[~ok], lo_[~ok])):
            idxs.append(rr)
            ca.append(ll)
            cb.append(np.full(len(rr), -1, np.int64))
    if n % 2:
        idxs.append(np.array([int(r[-1])], np.int64))
        ca.append(np.array([int(l[-1])], np.int64))
        cb.append(np.array([-1], np.int64))
    if not idxs:
        return (np.zeros(0, np.int64),) * 3
    idx = np.concatenate(idxs)
    cA = np.concatenate(ca)
    cB = np.concatenate(cb)
    o = np.argsort(idx, kind="stable")   # ascending HBM addresses
    return idx[o], cA[o], cB[o]


def _marshal(event_cell, event_row):
    ecell = np.asarray(event_cell).astype(np.int64)
    erow = np.asarray(event_row).astype(np.int64)
    order = np.argsort(ecell, kind="stable")
    scell = ecell[order]
    srow = erow[order].astype(np.int64)

    ntiles = NCELLS // 128
    bounds = np.searchsorted(scell, np.arange(ntiles + 1) * 128)
    counts = np.diff(bounds)

    # per-tile slot lists (events row-sorted, fixed-grid paired)
    tile_slots = []
    for t in range(ntiles):
        s, n = int(bounds[t]), int(counts[t])
        rr, ll = srow[s:s + n], scell[s:s + n] & 127
        ro = np.argsort(rr, kind="stable")
        tile_slots.append(_pair_tile(rr[ro], ll[ro]))
    k2 = np.array([max(1, -(-len(ts[0]) // 128)) for ts in tile_slots])

    # snake-deal tiles (sorted by chunk count desc) to cores
    deal = np.argsort(-k2, kind="stable")
    assign = [[] for _ in range(NCORES)]
    for rank, t in enumerate(deal):
        r = rank % (2 * NCORES)
        cidx = r if r < NCORES else 2 * NCORES - 1 - r
        assign[cidx].append(int(t))
    pos_tiles = [sorted(ts, key=lambda t: (-k2[t], t)) for ts in assign]
    perm = []
    for a, b in zip(range(TPC // 2), reversed(range(TPC // 2, TPC))):
        perm += [a, b]
    pos_tiles = [[ts[i] for i in perm] for ts in pos_tiles]
    S = np.max(np.stack([[k2[t] for t in ts] for ts in pos_tiles]), axis=0)
    S = S.astype(np.int64)
    S[-1] += (-int(S.sum())) % GCH
    nch = int(S.sum())
    off = np.concatenate([[0], np.cumsum(S)])

    in_maps = []
    for cidx in range(NCORES):
        slots_p = np.zeros(nch * 128, np.int16)     # idx 0 padding (row 0)
        lrel_p = np.full((2, nch * 128), -1.0, np.float32)
        for p, t in enumerate(pos_tiles[cidx]):
            idx, cA, cB = tile_slots[t]
            n = len(idx)
            base = int(off[p]) * 128
            slots_p[base:base + n] = idx.astype(np.int16)
            lrel_p[0, base:base + n] = cA
            lrel_p[1, base:base + n] = cB
        wr = slots_p.reshape(-1, GIDX).reshape(-1, GIDX // 16, 16)
        wr = wr.transpose(0, 2, 1).reshape(-1, 16, GIDX // 16)
        wr = np.concatenate(list(wr), axis=1)
        wr = np.tile(wr, (8, 1))
        lc = lrel_p.reshape(2, nch, 128).transpose(2, 0, 1)  # [128, 2, nch]
        in_maps.append({
            "rows_w": np.ascontiguousarray(wr),
            "lrel": np.ascontiguousarray(lc.astype(ml_dtypes.bfloat16)),
        })
    return in_maps, tuple(int(x) for x in S), pos_tiles


def kernel(table, event_cell, event_row, _want_trace=False):
    from concourse.bass_utils import run_bass_kernel_spmd

    tabbf = np.asarray(table, dtype=np.float32).astype(ml_dtypes.bfloat16)
    td = np.empty((NDELTA, TROWS, 2 * D), dtype=ml_dtypes.bfloat16)
    ar = np.arange(TROWS)
    for dlt in range(NDELTA):
        td[dlt, :, :D] = tabbf
        td[dlt, :, D:] = tabbf[np.minimum(ar + dlt, TROWS - 1)]
    td = np.ascontiguousarray(td.reshape(NDELTA * TROWS, 2 * D))

    in_maps, S, pos_tiles = _marshal(event_cell, event_row)
    for m in in_maps:
        m["tdall"] = td

    if S not in _compiled:
        _compiled[S] = _build(S)
    nc = _compiled[S]

    kw = {"trace": True} if _want_trace else {}
    res = run_bass_kernel_spmd(nc, in_maps, core_ids=list(range(NCORES)), **kw)
    full = np.empty((NCELLS // 128, 128, D), np.float32)
    for cidx in range(NCORES):
        co = np.asarray(res.results[cidx]["out"]).astype(np.float32)
        full[np.array(pos_tiles[cidx])] = co.transpose(1, 0, 2)
    out = full.reshape(B, W, H, L, D)
    if _want_trace:
        return out, res
    return out
